# revision 11
# baseline (speedup 1.0000x reference)
"""Trainium2 Bass kernel for nn_AdaptiveMask (dense MLP over upper-triangle gather).

Computation (reference):
    x_flat = x[:, iu0, iu1]                      # [B, M] strict-upper-tri gather
    h = relu(x_flat @ w1 + b1)                   # [B, H]
    m = sigmoid(h @ w2 + b2)                     # [B, M]
    W = scatter_sym(m); out = W * x              # [B, C, C]
    returns (out, m)

Strategy (8 NeuronCores):
  - GEMM1 is tensor-parallel over the hidden dim: core c holds w1[:, cols_c]
    and computes h_c = relu(x_flat @ w1_c) for the full batch.
  - The transposed activations hT_c ([128, 1280] bf16, only 0.33 MB) are
    AllGathered so every core holds the full hT; GEMM2 is then
    tensor-parallel over the OUTPUT columns — each core computes exact
    (unsummed) y[:, cols_c] = h @ w2[:, cols_c], so no output reduction,
    no y bounce traffic, and the collective channel carries ~2.6 MB once
    instead of ~40 MB of ReduceScatter traffic.
  - All matmuls in bf16 (fp32 PSUM accumulation); epilogue in fp32 straight
    from PSUM (sigmoid on ScalarE, gating multiplies on VectorE).
  - Weights/x are pre-permuted on host into per-partition-contiguous layouts
    so every weight DMA is a plain 2D transfer at line rate.
  - Biases are folded into the GEMMs (ones-column in x_flat picks up b1; a
    bias hidden-unit in core 7's hidden block injects b2, shared to all
    cores by the AllGather).
  - Host does only layout (triangle gather/scatter, pad, shard, permute,
    cast); every FLOP of the reference runs on device.
"""

import numpy as np
import ml_dtypes

import concourse.bass as bass
import concourse.bacc as bacc
import concourse.tile as tile
from concourse import mybir
from concourse import bass_utils

# ---- problem constants (must match reference.py's setup_inputs) ----
B, NCH = 128, 200
M, H = 19900, 9950
NCORES = 8
BL = B // NCORES

K1, KT1 = 19968, 156  # GEMM1 contraction (19900 data + 1 bias row + pad), k-tiles
HC = 1280             # per-core hidden slots
HTOT, KT2 = 10240, 80 # global padded hidden, GEMM2 k-tiles
H_PER = [1244] * 7 + [1242]
H_START = [0, 1244, 2488, 3732, 4976, 6220, 7464, 8708]
BIAS_SLOT = 1242      # block-7 local hidden slot for the b2 bias unit

MC = 2560             # per-core output columns (5 n-chunks of 512)
MC_REAL = [2560] * 7 + [19900 - 7 * 2560]  # real cols per core (core 7: 1980)
NCHK = 5              # n-chunks per core
KQ = 20               # k-tiles per w2 DMA quarter (4 quarters of 80)

W1_CHUNK = 4          # k-tiles per w1 DMA (1.3 MB bf16; short PE gaps keep HAM warm)
XT_SPLIT = 4          # xT loaded in 4 pieces so GEMM1 starts after the first

CDT = mybir.dt.bfloat16
NP_CDT = ml_dtypes.bfloat16
F32 = mybir.dt.float32

_IU = np.triu_indices(NCH, k=1)


def build_nc():
    nc = bacc.Bacc("TRN2", target_bir_lowering=False, debug=False, num_devices=NCORES)

    # host-permuted layouts: per-partition-contiguous (see prep_in_maps)
    xT = nc.dram_tensor("xT", [128, K1], CDT, kind="ExternalInput")
    w1 = nc.dram_tensor("w1", [128, KT1 * HC], CDT, kind="ExternalInput")
    w2 = nc.dram_tensor("w2", [128, KT2 * MC], CDT, kind="ExternalInput")
    xf = nc.dram_tensor("xf", [B, MC], F32, kind="ExternalInput")
    xtf = nc.dram_tensor("xtf", [B, MC], F32, kind="ExternalInput")
    om = nc.dram_tensor("om", [B, MC], F32, kind="ExternalOutput")
    ou = nc.dram_tensor("ou", [B, MC], F32, kind="ExternalOutput")
    ol = nc.dram_tensor("ol", [B, MC], F32, kind="ExternalOutput")

    with tile.TileContext(nc) as tc:
        with (
            tc.tile_pool(name="const", bufs=1) as constp,
            tc.tile_pool(name="hbuf", bufs=1) as hp,
            tc.tile_pool(name="dramc", bufs=1, space="DRAM") as dramc,
        ):
            ident_dram = nc.inline_tensor(np.eye(128, dtype=NP_CDT), name="ident")
            ident = constp.tile([128, 128], CDT)
            nc.scalar.dma_start(ident[:], ident_dram[:])

            # x_flat^T resident in SBUF: partition = k-within-tile, free = (ktile, batch)
            xT_sb = constp.tile([128, K1], CDT)
            xt_piece = K1 // XT_SPLIT
            for i in range(XT_SPLIT):
                nc.scalar.dma_start(
                    xT_sb[:, i * xt_piece : (i + 1) * xt_piece],
                    xT[:, i * xt_piece : (i + 1) * xt_piece],
                )

            h_sb = hp.tile([128, HC], CDT)    # [batch, hidden_local]
            hT_sb = hp.tile([128, HC], CDT)   # [hidden_local, batch] as 10 k-tiles
            hTf_sb = hp.tile([128, HTOT], CDT)  # full gathered hT, 80 k-tiles

            hT_bounce = dramc.tile([128, HC], CDT, name="hT_bounce")
            hT_all = dramc.tile(
                [NCORES * 128, HC], CDT, name="hT_all", addr_space="Shared"
            )

            # ---------------- GEMM1: h = relu(x_flat @ w1_c + b1_c) ----------------
            with (
                tc.tile_pool(name="w1p", bufs=4) as w1p,
                tc.tile_pool(name="ps1", bufs=1, space="PSUM") as ps1,
                tc.tile_pool(name="pst", bufs=2, space="PSUM") as pst,
            ):
                n_chunks1 = [(0, 512), (512, 1024), (1024, 1280)]
                ph = [
                    ps1.tile([128, n1 - n0], F32, tag=f"ph{i}", name=f"ph{i}")
                    for i, (n0, n1) in enumerate(n_chunks1)
                ]
                n_w1_chunks = (KT1 + W1_CHUNK - 1) // W1_CHUNK
                for c in range(n_w1_chunks):
                    kc = min(W1_CHUNK, KT1 - c * W1_CHUNK)
                    c0 = c * W1_CHUNK * HC
                    w1t = w1p.tile([128, kc * HC], CDT, tag="w1t", name=f"w1t{c}")
                    eng = nc.sync if c % 2 == 0 else nc.scalar
                    eng.dma_start(w1t[:], w1[:, c0 : c0 + kc * HC])
                    for j in range(kc):
                        k = c * W1_CHUNK + j
                        for i, (n0, n1) in enumerate(n_chunks1):
                            nc.tensor.matmul(
                                ph[i][:, :],
                                xT_sb[:, k * 128 : (k + 1) * 128],
                                w1t[:, j * HC + n0 : j * HC + n1],
                                start=(k == 0),
                                stop=(k == KT1 - 1),
                            )
                for i, (n0, n1) in enumerate(n_chunks1):
                    nc.scalar.activation(
                        h_sb[:, n0:n1], ph[i][:, :], mybir.ActivationFunctionType.Relu
                    )
                # transpose h -> hT via PE (GEMM2's stationary operand layout)
                for j in range(HC // 128):
                    pt = pst.tile([128, 128], CDT, tag="pt", name=f"pt{j}")
                    nc.tensor.transpose(pt[:], h_sb[:, j * 128 : (j + 1) * 128], ident[:])
                    nc.vector.tensor_copy(hT_sb[:, j * 128 : (j + 1) * 128], pt[:])

            # -------- AllGather hT (0.33 MB per core -> 2.6 MB, once) --------
            nc.scalar.dma_start(hT_bounce[:], hT_sb[:])
            nc.gpsimd.collective_compute(
                "AllGather",
                mybir.AluOpType.bypass,
                replica_groups=[list(range(NCORES))],
                ins=[hT_bounce[:].opt()],
                outs=[hT_all[:].opt()],
            )
            for c in range(NCORES):
                nc.scalar.dma_start(
                    hTf_sb[:, c * HC : (c + 1) * HC],
                    hT_all[c * 128 : (c + 1) * 128, :],
                )

            # ---- GEMM2 (exact column shard, no reduce) + fused epilogue ----
            with (
                tc.tile_pool(name="w2p", bufs=4) as w2p,
                tc.tile_pool(name="ps2", bufs=2, space="PSUM") as ps2,
                tc.tile_pool(name="ep", bufs=2) as ep,
            ):
                xfs = ep.tile([128, MC], F32, name="xfs", bufs=1)
                xtfs = ep.tile([128, MC], F32, name="xtfs", bufs=1)
                nc.sync.dma_start(xfs[:], xf[:])
                nc.sync.dma_start(xtfs[:], xtf[:])

                for n in range(NCHK):
                    pg = ps2.tile([128, 512], F32, tag="pg", name=f"pg{n}")
                    for q in range(4):
                        w2t = w2p.tile(
                            [128, KQ * 512], CDT, tag="w2t", name=f"w2t{n}_{q}"
                        )
                        off = (n * 4 + q) * (KQ * 512)
                        eng = nc.sync if q % 2 == 0 else nc.scalar
                        eng.dma_start(w2t[:], w2[:, off : off + KQ * 512])
                        for kk in range(KQ):
                            kg = q * KQ + kk
                            nc.tensor.matmul(
                                pg[:, :],
                                hTf_sb[:, kg * 128 : (kg + 1) * 128],
                                w2t[:, kk * 512 : (kk + 1) * 512],
                                start=(kg == 0),
                                stop=(kg == KT2 - 1),
                            )
                    ms = ep.tile([128, 512], F32, tag="ms", name=f"ms{n}")
                    nc.scalar.activation(
                        ms[:], pg[:, :], mybir.ActivationFunctionType.Sigmoid
                    )
                    us = ep.tile([128, 512], F32, tag="us", name=f"us{n}")
                    nc.vector.tensor_mul(us[:], ms[:], xfs[:, n * 512 : (n + 1) * 512])
                    ls = ep.tile([128, 512], F32, tag="ls", name=f"ls{n}")
                    nc.vector.tensor_mul(ls[:], ms[:], xtfs[:, n * 512 : (n + 1) * 512])
                    for t, dst in ((ms, om), (us, ou), (ls, ol)):
                        nc.scalar.dma_start(dst[:, n * 512 : (n + 1) * 512], t[:])

    nc.compile()
    return nc


def prep_in_maps(x, w1, b1, w2, b2):
    x = np.asarray(x)
    w1 = np.asarray(w1, dtype=np.float32)
    b1 = np.asarray(b1, dtype=np.float32)
    w2 = np.asarray(w2, dtype=np.float32)
    b2 = np.asarray(b2, dtype=np.float32)
    iu0, iu1 = _IU
    xfl = np.ascontiguousarray(x[:, iu0, iu1]).astype(np.float32)   # [B, M]
    xtfl = np.ascontiguousarray(x[:, iu1, iu0]).astype(np.float32)  # [B, M]

    # xT permuted: xT[p, k*128 + b] = x_aug^T[k*128 + p, b]
    xTa = np.zeros((K1, B), dtype=NP_CDT)
    xTa[:M] = xfl.T.astype(NP_CDT)
    xTa[M] = 1.0  # bias-ones row: picks up b1 (and block 7's b2 unit)
    xTp = np.ascontiguousarray(
        xTa.reshape(KT1, 128, B).transpose(1, 0, 2).reshape(128, K1)
    )

    # globally padded column space: 8 blocks of MC; core c owns block c
    MPAD = NCORES * MC
    xf_p = np.zeros((B, MPAD), np.float32)
    xf_p[:, :M] = xfl
    xtf_p = np.zeros((B, MPAD), np.float32)
    xtf_p[:, :M] = xtfl

    # w2 with globally padded hidden rows (8 blocks of HC) and padded cols,
    # b1/b2 folded: block-7 hidden slot BIAS_SLOT is the b2 bias unit.
    w2g = np.zeros((HTOT, MPAD), dtype=NP_CDT)
    for cb in range(NCORES):
        h0, hn = H_START[cb], H_PER[cb]
        w2g[cb * HC : cb * HC + hn, :M] = w2[h0 : h0 + hn, :].astype(NP_CDT)
    w2g[7 * HC + BIAS_SLOT, :M] = b2.astype(NP_CDT)

    in_maps = []
    for c in range(NCORES):
        h0, hn = H_START[c], H_PER[c]
        w1c = np.zeros((K1, HC), dtype=NP_CDT)
        w1c[:M, :hn] = w1[:, h0 : h0 + hn].astype(NP_CDT)
        w1c[M, :hn] = b1[h0 : h0 + hn].astype(NP_CDT)
        if c == NCORES - 1:
            w1c[M, BIAS_SLOT] = 1.0  # h[:, BIAS_SLOT] = relu(1*1) = 1 on core 7 only
        # permute: w1p[p, k*HC + f] = w1c[k*128 + p, f]
        w1p = np.ascontiguousarray(
            w1c.reshape(KT1, 128, HC).transpose(1, 0, 2).reshape(128, KT1 * HC)
        )
        # w2 shard: all hidden rows, own column block; permuted per (n, q):
        # w2p[p, ((n*4+q)*KQ + kk)*512 + f] = w2g[(q*KQ+kk)*128 + p, c*MC + n*512 + f]
        shard = w2g[:, c * MC : (c + 1) * MC].reshape(KT2, 128, MC)
        blocks = []
        for n in range(NCHK):
            for q in range(4):
                blk = shard[q * KQ : (q + 1) * KQ, :, n * 512 : (n + 1) * 512]
                blocks.append(blk.transpose(1, 0, 2).reshape(128, KQ * 512))
        w2p = np.ascontiguousarray(np.concatenate(blocks, axis=1))
        in_maps.append(
            {
                "xT": xTp,
                "w1": w1p,
                "w2": w2p,
                "xf": np.ascontiguousarray(xf_p[:, c * MC : (c + 1) * MC]),
                "xtf": np.ascontiguousarray(xtf_p[:, c * MC : (c + 1) * MC]),
            }
        )
    return in_maps


def assemble(results):
    m = np.concatenate(
        [results[c]["om"][:, : MC_REAL[c]] for c in range(NCORES)], axis=1
    )
    u = np.concatenate(
        [results[c]["ou"][:, : MC_REAL[c]] for c in range(NCORES)], axis=1
    )
    l = np.concatenate(
        [results[c]["ol"][:, : MC_REAL[c]] for c in range(NCORES)], axis=1
    )
    iu0, iu1 = _IU
    out = np.zeros((B, NCH, NCH), np.float32)
    out[:, iu0, iu1] = u
    out[:, iu1, iu0] = l
    return out.astype(np.float32), m.astype(np.float32)


_NC_CACHE = None


def kernel(x, w1, b1, w2, b2, _trace=False):
    global _NC_CACHE
    in_maps = prep_in_maps(x, w1, b1, w2, b2)
    if _NC_CACHE is None:
        _NC_CACHE = build_nc()
    res = bass_utils.run_bass_kernel_spmd(
        _NC_CACHE, in_maps, core_ids=list(range(NCORES)), trace=_trace
    )
    out = assemble(res.results)
    if _trace:
        return out, res
    return out


# revision 13
# speedup vs baseline: 1.2437x; 1.2437x over previous
"""Trainium2 Bass kernel for nn_AdaptiveMask (dense MLP over upper-triangle gather).

Computation (reference):
    x_flat = x[:, iu0, iu1]                      # [B, M] strict-upper-tri gather
    h = relu(x_flat @ w1 + b1)                   # [B, H]
    m = sigmoid(h @ w2 + b2)                     # [B, M]
    W = scatter_sym(m); out = W * x              # [B, C, C]
    returns (out, m)

Strategy (8 NeuronCores):
  - GEMM1 is tensor-parallel over the hidden dim: core c holds w1[:, cols_c]
    and computes h_c = relu(x_flat @ w1_c) for the full batch.
  - The transposed activations hT_c ([128, 1280] bf16, only 0.33 MB) are
    AllGathered so every core holds the full hT; GEMM2 is then
    tensor-parallel over the OUTPUT columns — each core computes exact
    (unsummed) y[:, cols_c] = h @ w2[:, cols_c], so no output reduction,
    no y bounce traffic, and the collective channel carries ~2.6 MB once
    instead of ~40 MB of ReduceScatter traffic.
  - All matmuls in bf16 (fp32 PSUM accumulation); epilogue in fp32 straight
    from PSUM (sigmoid on ScalarE, gating multiplies on VectorE).
  - Weights/x are pre-permuted on host into per-partition-contiguous layouts
    so every weight DMA is a plain 2D transfer at line rate.
  - Biases are folded into the GEMMs (ones-column in x_flat picks up b1; a
    bias hidden-unit in core 7's hidden block injects b2, shared to all
    cores by the AllGather).
  - Host does only layout (triangle gather/scatter, pad, shard, permute,
    cast); every FLOP of the reference runs on device.
"""

import numpy as np
import ml_dtypes

import concourse.bass as bass
import concourse.bacc as bacc
import concourse.tile as tile
from concourse import mybir
from concourse import bass_utils

# ---- problem constants (must match reference.py's setup_inputs) ----
B, NCH = 128, 200
M, H = 19900, 9950
NCORES = 8
BL = B // NCORES

K1, KT1 = 19968, 156  # GEMM1 contraction (19900 data + 1 bias row + pad), k-tiles
HC = 1280             # per-core hidden slots
HTOT, KT2 = 10240, 80 # global padded hidden, GEMM2 k-tiles
H_PER = [1244] * 7 + [1242]
H_START = [0, 1244, 2488, 3732, 4976, 6220, 7464, 8708]
BIAS_SLOT = 1242      # block-7 local hidden slot for the b2 bias unit

MC = 2560             # per-core output columns (5 n-chunks of 512)
MC_REAL = [2560] * 7 + [19900 - 7 * 2560]  # real cols per core (core 7: 1980)
NCHK = 5              # n-chunks per core
KQ = 20               # k-tiles per w2 DMA quarter (4 quarters of 80)

W1_CHUNK = 4          # k-tiles per w1 DMA (1.3 MB bf16; short PE gaps keep HAM warm)
XT_SPLIT = 4          # xT loaded in 4 pieces so GEMM1 starts after the first

CDT = mybir.dt.bfloat16
NP_CDT = ml_dtypes.bfloat16
F32 = mybir.dt.float32

_IU = np.triu_indices(NCH, k=1)


def build_nc():
    nc = bacc.Bacc("TRN2", target_bir_lowering=False, debug=False, num_devices=NCORES)

    # host-permuted layouts: per-partition-contiguous (see prep_in_maps)
    xT = nc.dram_tensor("xT", [128, K1], CDT, kind="ExternalInput")
    w1 = nc.dram_tensor("w1", [128, KT1 * HC], CDT, kind="ExternalInput")
    w2 = nc.dram_tensor("w2", [128, KT2 * MC], CDT, kind="ExternalInput")
    xf = nc.dram_tensor("xf", [B, MC], F32, kind="ExternalInput")
    xtf = nc.dram_tensor("xtf", [B, MC], F32, kind="ExternalInput")
    om = nc.dram_tensor("om", [B, MC], F32, kind="ExternalOutput")
    ou = nc.dram_tensor("ou", [B, MC], F32, kind="ExternalOutput")
    ol = nc.dram_tensor("ol", [B, MC], F32, kind="ExternalOutput")

    with tile.TileContext(nc) as tc:
        with (
            tc.tile_pool(name="const", bufs=1) as constp,
            tc.tile_pool(name="hbuf", bufs=1) as hp,
            tc.tile_pool(name="dramc", bufs=1, space="DRAM") as dramc,
        ):
            ident_dram = nc.inline_tensor(np.eye(128, dtype=NP_CDT), name="ident")
            ident = constp.tile([128, 128], CDT)
            nc.scalar.dma_start(ident[:], ident_dram[:])

            # x_flat^T resident in SBUF: partition = k-within-tile, free = (ktile, batch)
            xT_sb = constp.tile([128, K1], CDT)
            xt_piece = K1 // XT_SPLIT
            for i in range(XT_SPLIT):
                nc.scalar.dma_start(
                    xT_sb[:, i * xt_piece : (i + 1) * xt_piece],
                    xT[:, i * xt_piece : (i + 1) * xt_piece],
                )

            h_sb = hp.tile([128, HC], CDT)    # [batch, hidden_local]
            hT_sb = hp.tile([128, HC], CDT)   # [hidden_local, batch] as 10 k-tiles
            hTf_sb = hp.tile([128, HTOT], CDT)  # full gathered hT, 80 k-tiles

            hT_bounce = dramc.tile([128, HC], CDT, name="hT_bounce")
            hT_all = dramc.tile(
                [NCORES * 128, HC], CDT, name="hT_all", addr_space="Shared"
            )

            # warm up the collective path while GEMM1 runs: the first ncfw
            # triggers pay a large per-rank init cost (v4 measured 132 us for
            # the first AllGather; v3's warmed collectives ran in 14-27 us)
            cc_wi = dramc.tile([128, 128], CDT, name="cc_wi")
            cc_wo = [
                dramc.tile(
                    [NCORES * 128, 128], CDT, name=f"cc_wo{i}", addr_space="Shared"
                )
                for i in range(2)
            ]
            for i in range(2):
                nc.gpsimd.collective_compute(
                    "AllGather",
                    mybir.AluOpType.bypass,
                    replica_groups=[list(range(NCORES))],
                    ins=[cc_wi[:].opt()],
                    outs=[cc_wo[i][:].opt()],
                )

            # ---------------- GEMM1: h = relu(x_flat @ w1_c + b1_c) ----------------
            with (
                tc.tile_pool(name="w1p", bufs=4) as w1p,
                tc.tile_pool(name="ps1", bufs=1, space="PSUM") as ps1,
                tc.tile_pool(name="pst", bufs=2, space="PSUM") as pst,
            ):
                n_chunks1 = [(0, 512), (512, 1024), (1024, 1280)]
                ph = [
                    ps1.tile([128, n1 - n0], F32, tag=f"ph{i}", name=f"ph{i}")
                    for i, (n0, n1) in enumerate(n_chunks1)
                ]
                n_w1_chunks = (KT1 + W1_CHUNK - 1) // W1_CHUNK
                for c in range(n_w1_chunks):
                    kc = min(W1_CHUNK, KT1 - c * W1_CHUNK)
                    c0 = c * W1_CHUNK * HC
                    w1t = w1p.tile([128, kc * HC], CDT, tag="w1t", name=f"w1t{c}")
                    eng = nc.sync if c % 2 == 0 else nc.scalar
                    eng.dma_start(w1t[:], w1[:, c0 : c0 + kc * HC])
                    for j in range(kc):
                        k = c * W1_CHUNK + j
                        for i, (n0, n1) in enumerate(n_chunks1):
                            nc.tensor.matmul(
                                ph[i][:, :],
                                xT_sb[:, k * 128 : (k + 1) * 128],
                                w1t[:, j * HC + n0 : j * HC + n1],
                                start=(k == 0),
                                stop=(k == KT1 - 1),
                            )
                for i, (n0, n1) in enumerate(n_chunks1):
                    nc.scalar.activation(
                        h_sb[:, n0:n1], ph[i][:, :], mybir.ActivationFunctionType.Relu
                    )
                # transpose h -> hT via PE (GEMM2's stationary operand layout)
                for j in range(HC // 128):
                    pt = pst.tile([128, 128], CDT, tag="pt", name=f"pt{j}")
                    nc.tensor.transpose(pt[:], h_sb[:, j * 128 : (j + 1) * 128], ident[:])
                    nc.vector.tensor_copy(hT_sb[:, j * 128 : (j + 1) * 128], pt[:])

            # -------- AllGather hT (0.33 MB per core -> 2.6 MB, once) --------
            # gpsimd owns the collective AND the gather-dependent loads, so no
            # other sequencer ever waits on the AllGather (sync/scalar keep
            # streaming w2/xf during it).
            nc.gpsimd.dma_start(hT_bounce[:], hT_sb[:])
            nc.gpsimd.collective_compute(
                "AllGather",
                mybir.AluOpType.bypass,
                replica_groups=[list(range(NCORES))],
                ins=[hT_bounce[:].opt()],
                outs=[hT_all[:].opt()],
            )
            for c in range(NCORES):
                nc.gpsimd.dma_start(
                    hTf_sb[:, c * HC : (c + 1) * HC],
                    hT_all[c * 128 : (c + 1) * 128, :],
                )

            # ---- GEMM2 (exact column shard, no reduce) + fused epilogue ----
            with (
                tc.tile_pool(name="w2p", bufs=4) as w2p,
                tc.tile_pool(name="ps2", bufs=2, space="PSUM") as ps2,
                tc.tile_pool(name="ep", bufs=2) as ep,
            ):
                xfs = ep.tile([128, MC], F32, name="xfs", bufs=1)
                xtfs = ep.tile([128, MC], F32, name="xtfs", bufs=1)
                nc.sync.dma_start(xfs[:], xf[:])
                nc.sync.dma_start(xtfs[:], xtf[:])

                for n in range(NCHK):
                    pg = ps2.tile([128, 512], F32, tag="pg", name=f"pg{n}")
                    for q in range(4):
                        w2t = w2p.tile(
                            [128, KQ * 512], CDT, tag="w2t", name=f"w2t{n}_{q}"
                        )
                        off = (n * 4 + q) * (KQ * 512)
                        eng = nc.sync if q % 2 == 0 else nc.scalar
                        eng.dma_start(w2t[:], w2[:, off : off + KQ * 512])
                        for kk in range(KQ):
                            kg = q * KQ + kk
                            nc.tensor.matmul(
                                pg[:, :],
                                hTf_sb[:, kg * 128 : (kg + 1) * 128],
                                w2t[:, kk * 512 : (kk + 1) * 512],
                                start=(kg == 0),
                                stop=(kg == KT2 - 1),
                            )
                    ms = ep.tile([128, 512], F32, tag="ms", name=f"ms{n}")
                    nc.scalar.activation(
                        ms[:], pg[:, :], mybir.ActivationFunctionType.Sigmoid
                    )
                    us = ep.tile([128, 512], F32, tag="us", name=f"us{n}")
                    nc.vector.tensor_mul(us[:], ms[:], xfs[:, n * 512 : (n + 1) * 512])
                    ls = ep.tile([128, 512], F32, tag="ls", name=f"ls{n}")
                    nc.vector.tensor_mul(ls[:], ms[:], xtfs[:, n * 512 : (n + 1) * 512])
                    for t, dst in ((ms, om), (us, ou), (ls, ol)):
                        nc.scalar.dma_start(dst[:, n * 512 : (n + 1) * 512], t[:])

    nc.compile()
    return nc


def prep_in_maps(x, w1, b1, w2, b2):
    x = np.asarray(x)
    w1 = np.asarray(w1, dtype=np.float32)
    b1 = np.asarray(b1, dtype=np.float32)
    w2 = np.asarray(w2, dtype=np.float32)
    b2 = np.asarray(b2, dtype=np.float32)
    iu0, iu1 = _IU
    xfl = np.ascontiguousarray(x[:, iu0, iu1]).astype(np.float32)   # [B, M]
    xtfl = np.ascontiguousarray(x[:, iu1, iu0]).astype(np.float32)  # [B, M]

    # xT permuted: xT[p, k*128 + b] = x_aug^T[k*128 + p, b]
    xTa = np.zeros((K1, B), dtype=NP_CDT)
    xTa[:M] = xfl.T.astype(NP_CDT)
    xTa[M] = 1.0  # bias-ones row: picks up b1 (and block 7's b2 unit)
    xTp = np.ascontiguousarray(
        xTa.reshape(KT1, 128, B).transpose(1, 0, 2).reshape(128, K1)
    )

    # globally padded column space: 8 blocks of MC; core c owns block c
    MPAD = NCORES * MC
    xf_p = np.zeros((B, MPAD), np.float32)
    xf_p[:, :M] = xfl
    xtf_p = np.zeros((B, MPAD), np.float32)
    xtf_p[:, :M] = xtfl

    # w2 with globally padded hidden rows (8 blocks of HC) and padded cols,
    # b1/b2 folded: block-7 hidden slot BIAS_SLOT is the b2 bias unit.
    w2g = np.zeros((HTOT, MPAD), dtype=NP_CDT)
    for cb in range(NCORES):
        h0, hn = H_START[cb], H_PER[cb]
        w2g[cb * HC : cb * HC + hn, :M] = w2[h0 : h0 + hn, :].astype(NP_CDT)
    w2g[7 * HC + BIAS_SLOT, :M] = b2.astype(NP_CDT)

    in_maps = []
    for c in range(NCORES):
        h0, hn = H_START[c], H_PER[c]
        w1c = np.zeros((K1, HC), dtype=NP_CDT)
        w1c[:M, :hn] = w1[:, h0 : h0 + hn].astype(NP_CDT)
        w1c[M, :hn] = b1[h0 : h0 + hn].astype(NP_CDT)
        if c == NCORES - 1:
            w1c[M, BIAS_SLOT] = 1.0  # h[:, BIAS_SLOT] = relu(1*1) = 1 on core 7 only
        # permute: w1p[p, k*HC + f] = w1c[k*128 + p, f]
        w1p = np.ascontiguousarray(
            w1c.reshape(KT1, 128, HC).transpose(1, 0, 2).reshape(128, KT1 * HC)
        )
        # w2 shard: all hidden rows, own column block; permuted per (n, q):
        # w2p[p, ((n*4+q)*KQ + kk)*512 + f] = w2g[(q*KQ+kk)*128 + p, c*MC + n*512 + f]
        shard = w2g[:, c * MC : (c + 1) * MC].reshape(KT2, 128, MC)
        blocks = []
        for n in range(NCHK):
            for q in range(4):
                blk = shard[q * KQ : (q + 1) * KQ, :, n * 512 : (n + 1) * 512]
                blocks.append(blk.transpose(1, 0, 2).reshape(128, KQ * 512))
        w2p = np.ascontiguousarray(np.concatenate(blocks, axis=1))
        in_maps.append(
            {
                "xT": xTp,
                "w1": w1p,
                "w2": w2p,
                "xf": np.ascontiguousarray(xf_p[:, c * MC : (c + 1) * MC]),
                "xtf": np.ascontiguousarray(xtf_p[:, c * MC : (c + 1) * MC]),
            }
        )
    return in_maps


def assemble(results):
    m = np.concatenate(
        [results[c]["om"][:, : MC_REAL[c]] for c in range(NCORES)], axis=1
    )
    u = np.concatenate(
        [results[c]["ou"][:, : MC_REAL[c]] for c in range(NCORES)], axis=1
    )
    l = np.concatenate(
        [results[c]["ol"][:, : MC_REAL[c]] for c in range(NCORES)], axis=1
    )
    iu0, iu1 = _IU
    out = np.zeros((B, NCH, NCH), np.float32)
    out[:, iu0, iu1] = u
    out[:, iu1, iu0] = l
    return out.astype(np.float32), m.astype(np.float32)


_NC_CACHE = None


def kernel(x, w1, b1, w2, b2, _trace=False):
    global _NC_CACHE
    in_maps = prep_in_maps(x, w1, b1, w2, b2)
    if _NC_CACHE is None:
        _NC_CACHE = build_nc()
    res = bass_utils.run_bass_kernel_spmd(
        _NC_CACHE, in_maps, core_ids=list(range(NCORES)), trace=_trace
    )
    out = assemble(res.results)
    if _trace:
        return out, res
    return out
